# revision 20
# baseline (speedup 1.0000x reference)
"""Trainium2 Bass kernel for nn_LorentzGraphConvolution.

Row-sharded across 8 NeuronCores: core c owns rows [c*1536, (c+1)*1536) of
the attention matrix / output. Every core redundantly computes the tiny
linear phase (h, k for all N; q for its local rows) from broadcast inputs,
so no collectives are needed; the only large input is each core's
[1536, 12288] slab of adj.

Key layout choices (per core):
  - att is computed TRANSPOSED (attT[j, i] tiles, j on partitions) via
    matmul(lhsT=kT block, rhs=qmT chunk) so the support matmul
    (contraction over j) consumes attT tiles directly with no transpose
    of att.
  - adj is cast f32->bf16 during the HBM DMA (SWDGE) and transposed to
    adjT in 128x128 blocks with the 2-byte xbar DMA-transpose, costing no
    engine time.
  - All matmuls run in bf16 (validated: ~8e-4 scaled output error); the
    Lorentz normalizations run in f32 on DVE/ACT from PSUM.
"""

import math
import os
import sys
from contextlib import ExitStack

for _p in ("/opt/trn_rl_repo", "/root/.axon_site/_ro/trn_rl_repo", "/root/.axon_site"):
    if os.path.isdir(_p) and _p not in sys.path:
        sys.path.insert(0, _p)

import ml_dtypes
import numpy as np

import concourse.bass as bass
import concourse.tile as tile
from concourse import bacc, bass_utils, masks, mybir

DT = mybir.dt
F32 = DT.float32
BF16 = DT.bfloat16
AF = mybir.ActivationFunctionType
ALU = mybir.AluOpType

N_FULL = 12288
D = 64
N_CORES = 8
R_FULL = N_FULL // N_CORES  # 1536 rows per core


def emit(tc, io, nn, rr, esc, esc_q, esc_k, sig_scale, sig_bias):
    """Emit the per-core Tile program.

    io: dict of bass.AP DRAM tensors:
      adj  f32  [rr, nn]      core's row slab of adj
      xT   bf16 [65, nn]      x transposed, row 64 = ones (bias row for W)
      xqT  bf16 [65, rr]      local slice of xT
      wT   bf16 [65, 64]      [W.T; b]
      wqT  bf16 [64, 64]      Wq.T
      wkT  bf16 [64, 64]      Wk.T
      bq   bf16 [1, 64]
      bk   bf16 [1, 64]
      out  f32  [rr, 64]
    """
    nc = tc.nc
    TJ = nn // 128          # global 128-row tiles
    TL = rr // 128          # local 128-row tiles
    IC = min(512, rr)       # i-chunk width (attention column block per core)
    NIC = rr // IC
    ICT = IC // 128         # 128-subtiles per i-chunk
    SW = min(2048, nn)      # adj strip width
    NSG = nn // SW
    JPG = SW // 128         # j tiles per strip group
    assert TJ % 2 == 0 and TL % 2 == 0 and rr % IC == 0 and nn % SW == 0

    ctx = ExitStack()

    const = ctx.enter_context(tc.tile_pool(name="const", bufs=1))
    persist = ctx.enter_context(tc.tile_pool(name="persist", bufs=1))
    flat = ctx.enter_context(tc.tile_pool(name="flat", bufs=2))
    psum_lin = ctx.enter_context(tc.tile_pool(name="psum_lin", bufs=2, space="PSUM"))
    psum_att = ctx.enter_context(tc.tile_pool(name="psum_att", bufs=3, space="PSUM"))
    psum_sup = ctx.enter_context(tc.tile_pool(name="psum_sup", bufs=2, space="PSUM"))
    small = ctx.enter_context(tc.tile_pool(name="small", bufs=8))
    wide = ctx.enter_context(tc.tile_pool(name="wide", bufs=2))
    oneshot = ctx.enter_context(tc.tile_pool(name="oneshot", bufs=1))
    strip_pool = ctx.enter_context(tc.tile_pool(name="strips", bufs=2 * ICT))
    adjT_pool = ctx.enter_context(tc.tile_pool(name="adjT", bufs=8))
    sig_pool = ctx.enter_context(tc.tile_pool(name="sig", bufs=4))
    attm_pool = ctx.enter_context(tc.tile_pool(name="attm", bufs=4))
    out_pool = ctx.enter_context(tc.tile_pool(name="outp", bufs=4))

    # ---- constants / small inputs -------------------------------------
    # xT shares the 2-slot "flat" pool: dead once phase A's matmuls finish,
    # freeing its slot for kpad.
    xT_s = flat.tile([65, nn], BF16, tag="flat")
    nc.sync.dma_start(xT_s[:], io["xT"][:])
    xqT_s = const.tile([65, rr], BF16)
    nc.sync.dma_start(xqT_s[:], io["xqT"][:])
    wT_s = const.tile([65, 64], BF16)
    nc.sync.dma_start(wT_s[:], io["wT"][:])
    wqT_s = const.tile([64, 64], BF16)
    nc.sync.dma_start(wqT_s[:], io["wqT"][:])
    wkT_s = const.tile([64, 64], BF16)
    nc.sync.dma_start(wkT_s[:], io["wkT"][:])
    bq_s = const.tile([1, 64], BF16)
    nc.sync.dma_start(bq_s[:], io["bq"][:])
    bk_s = const.tile([1, 64], BF16)
    nc.sync.dma_start(bk_s[:], io["bk"][:])
    ones_col = const.tile([1, 128], BF16)
    nc.vector.memset(ones_col[:], 1.0)
    ident = const.tile([64, 64], F32)
    masks.make_identity(nc, ident[:])
    sig_bias_ap = const.tile([128, 1], F32)
    nc.vector.memset(sig_bias_ap[:], sig_bias)

    # persistent per-core tensors. "pad" slabs put tile t's 64 features in
    # cols [t*128, t*128+64) so a 128x128 block DMA-transpose lands the
    # features at partitions 0:64; pad regions are never read.
    hpad = persist.tile([128, TJ * 128], BF16)      # h, natural row tiles
    kT_flat = persist.tile([128, TJ * 128], BF16)   # k^T, rows 0:64 valid
    qmT_flat = persist.tile([128, TL * 128], BF16)  # qm^T, rows 0:64 valid

    hpad3 = hpad.rearrange("p (t c) -> p t c", c=128)
    nc.vector.memset(hpad[:], 0.0)

    # ---- batched LorentzLinear ---------------------------------------
    def lorentz_linear(tiles, lhsT_fn, rhs_w, bias_row, esc_, neg, wr_slab3, pad):
        """Matmul + Lorentz renormalization for a batch of row tiles.

        wr_slab3: [128, T, c] destination view (c = 64 dense or 128 padded);
        writes time into [:, t, 0] and scaled spatial into [:, t, 1:64].
        """
        nb = len(tiles)
        ps = psum_lin.tile([128, nb * 64], F32, tag="linpsum")
        ps3 = ps.rearrange("p (t d) -> p t d", d=64)
        for u, t in enumerate(tiles):
            o = ps[:, u * 64:(u + 1) * 64]
            if bias_row is None:
                nc.tensor.matmul(o, lhsT_fn(t), rhs_w, start=True, stop=True)
            else:
                nc.tensor.matmul(o, lhsT_fn(t), rhs_w, start=True, stop=False)
                nc.tensor.matmul(o, ones_col[:], bias_row, start=False, stop=True)
        sg = small.tile([128, nb], F32, tag="nsg")
        nc.scalar.activation(sg[:], ps3[:, :, 0], AF.Sigmoid)
        time = small.tile([128, nb], F32, tag="ntime")
        a, c0 = (-esc_, -1.1) if neg else (esc_, 1.1)
        nc.vector.tensor_scalar(time[:], sg[:], a, c0, ALU.mult, ALU.add)
        sqf = wide.tile([128, nb * 64], F32, tag="nsqf")
        nc.scalar.activation(sqf[:], ps[:], AF.Square)
        sqf3 = sqf.rearrange("p (t d) -> p t d", d=64)
        tot = small.tile([128, nb], F32, tag="ntot")
        nc.vector.tensor_reduce(tot[:], sqf3[:], axis=mybir.AxisListType.X,
                                op=ALU.add)
        p0sq = small.tile([128, nb], F32, tag="np0")
        nc.vector.tensor_copy(p0sq[:], sqf3[:, :, 0])
        sq = small.tile([128, nb], F32, tag="nsq")
        # sq = tot - p0sq  (spatial sum of squares)
        nc.vector.scalar_tensor_tensor(sq[:], p0sq[:], -1.0, tot[:],
                                       ALU.mult, ALU.add)
        sqc = small.tile([128, nb], F32, tag="nsqc")
        nc.vector.tensor_scalar_max(sqc[:], sq[:], 1e-8)
        rcp = small.tile([128, nb], F32, tag="nrcp")
        nc.vector.reciprocal(rcp[:], sqc[:])
        t2 = small.tile([128, nb], F32, tag="nt2")
        nc.vector.tensor_tensor(t2[:], time[:], time[:], ALU.mult)
        sval = small.tile([128, nb], F32, tag="nsv")
        # s = (t^2 - 1) * rcp
        nc.vector.scalar_tensor_tensor(sval[:], t2[:], -1.0, rcp[:],
                                       ALU.add, ALU.mult)
        sqs = small.tile([128, nb], F32, tag="nsqs")
        nc.scalar.activation(sqs[:], sval[:], AF.Sqrt)
        t0 = tiles[0]
        nc.vector.tensor_copy(wr_slab3[:, t0:t0 + nb, 0], time[:])
        for u, t in enumerate(tiles):
            nc.vector.tensor_scalar_mul(wr_slab3[:, t, 1:64],
                                        ps3[:, u, 1:64], sqs[:, u:u + 1])

    def batches(total):
        return [list(range(s, min(s + 8, total))) for s in range(0, total, 8)]

    # ---- phase A: h (all rows) ---------------------------------------
    for bt in batches(TJ):
        lorentz_linear(bt, lambda t: xT_s[:, t * 128:(t + 1) * 128],
                       wT_s[:], None, esc, False, hpad3, False)
    hT_flat = flat.tile([128, TJ * 128], BF16, tag="flat")
    for t in range(TJ):
        nc.sync.dma_start(hT_flat[:, t * 128:(t + 1) * 128],
                          hpad[:, t * 128:(t + 1) * 128], transpose=True)

    # ---- phase B: k (all rows) ---------------------------------------
    kpad = flat.tile([128, TJ * 128], BF16, tag="flat")
    kpad3 = kpad.rearrange("p (t c) -> p t c", c=128)
    nc.vector.memset(kpad[:], 0.0)

    def h_lhsT(t):
        return hT_flat[0:64, t * 128:(t + 1) * 128]

    for bt in batches(TJ):
        lorentz_linear(bt, h_lhsT, wkT_s[:], bk_s[:], esc_k, False,
                       kpad3, False)
    for t in range(TJ):
        nc.sync.dma_start(kT_flat[:, t * 128:(t + 1) * 128],
                          kpad[:, t * 128:(t + 1) * 128], transpose=True)

    # ---- phase Bq: hq + qm (local rows) ------------------------------
    hqpad = oneshot.tile([128, TL * 128], BF16, tag="hq")
    hqpad3 = hqpad.rearrange("p (t c) -> p t c", c=128)
    nc.vector.memset(hqpad[:], 0.0)
    for bt in batches(TL):
        lorentz_linear(bt, lambda t: xqT_s[:, t * 128:(t + 1) * 128],
                       wT_s[:], None, esc, False, hqpad3, False)
    hqT_flat = oneshot.tile([128, TL * 128], BF16, tag="hqT")
    for t in range(TL):
        nc.sync.dma_start(hqT_flat[:, t * 128:(t + 1) * 128],
                          hqpad[:, t * 128:(t + 1) * 128], transpose=True)

    qm_pad = oneshot.tile([128, TL * 128], BF16, tag="qmpad")
    qm_pad3 = qm_pad.rearrange("p (t c) -> p t c", c=128)
    nc.vector.memset(qm_pad[:], 0.0)

    def hq_lhsT(t):
        return hqT_flat[0:64, t * 128:(t + 1) * 128]

    for bt in batches(TL):
        lorentz_linear(bt, hq_lhsT, wqT_s[:], bq_s[:], esc_q, True,
                       qm_pad3, True)
    for t in range(TL):
        nc.sync.dma_start(qmT_flat[:, t * 128:(t + 1) * 128],
                          qm_pad[:, t * 128:(t + 1) * 128], transpose=True)

    # ---- phase C: attention + support --------------------------------
    for c in range(NIC):
        supT = psum_sup.tile([64, IC], F32, tag="supT")
        for g in range(NSG):
            strips = []
            for s in range(ICT):
                st = strip_pool.tile([128, SW], BF16, tag="strip")
                r0 = c * IC + s * 128
                nc.gpsimd.dma_start(st[:], io["adj"][r0:r0 + 128,
                                                     g * SW:(g + 1) * SW])
                strips.append(st)
            for jl in range(JPG):
                j = g * JPG + jl
                adjT = adjT_pool.tile([128, IC], BF16, tag="adjT")
                for s in range(ICT):
                    nc.sync.dma_start(adjT[:, s * 128:(s + 1) * 128],
                                      strips[s][:, jl * 128:(jl + 1) * 128],
                                      transpose=True)
                attT = psum_att.tile([128, IC], F32, tag="attT")
                lhsT_k = kT_flat[0:64, j * 128:(j + 1) * 128]
                nc.tensor.matmul(attT[:], lhsT_k,
                                 qmT_flat[0:64, c * IC:(c + 1) * IC],
                                 start=True, stop=True)
                sig = sig_pool.tile([128, IC], BF16, tag="sig")
                nc.scalar.activation(sig[:], attT[:], AF.Sigmoid,
                                     bias=sig_bias_ap[:], scale=sig_scale)
                attm = attm_pool.tile([128, IC], BF16, tag="attm")
                nc.vector.tensor_mul(attm[:], sig[:], adjT[:])
                nc.tensor.matmul(supT[:], hpad[:, j * 128:j * 128 + 64],
                                 attm[:], start=(j == 0), stop=(j == TJ - 1))
        # normalize + write out this i-chunk
        supTs = wide.tile([64, IC], F32, tag="supTs")
        nc.vector.tensor_copy(supTs[:], supT[:])
        for s in range(ICT):
            supn = psum_lin.tile([128, 64], F32, tag="linpsum")
            nc.tensor.transpose(supn[:], supTs[:, s * 128:(s + 1) * 128],
                                ident[:])
            sq64 = out_pool.tile([128, 64], F32, tag="sq64")
            nc.scalar.activation(sq64[:], supn[:], AF.Square)
            tot = small.tile([128, 1], F32, tag="ftot")
            nc.vector.tensor_reduce(tot[:], sq64[:], axis=mybir.AxisListType.X,
                                    op=ALU.add)
            inner = small.tile([128, 1], F32, tag="finner")
            # inner = tot - 2*s0^2  (= -s0^2 + sum_{d>=1} s_d^2)
            nc.vector.scalar_tensor_tensor(inner[:], sq64[:, 0:1], -2.0,
                                           tot[:], ALU.mult, ALU.add)
            negv = small.tile([128, 1], F32, tag="fneg")
            nc.vector.tensor_scalar_mul(negv[:], inner[:], -1.0)
            absv = small.tile([128, 1], F32, tag="fabs")
            nc.vector.tensor_tensor(absv[:], inner[:], negv[:], ALU.max)
            clipv = small.tile([128, 1], F32, tag="fclip")
            nc.vector.tensor_scalar_max(clipv[:], absv[:], 1e-8)
            rcp = small.tile([128, 1], F32, tag="frcp")
            nc.vector.reciprocal(rcp[:], clipv[:])
            rs = small.tile([128, 1], F32, tag="frs")
            nc.scalar.activation(rs[:], rcp[:], AF.Sqrt)
            o = out_pool.tile([128, 64], F32, tag="otile")
            nc.vector.tensor_scalar_mul(o[:], supn[:], rs[:])
            r0 = c * IC + s * 128
            nc.sync.dma_start(io["out"][r0:r0 + 128, :], o[:])

    ctx.close()


def build(nn, rr, esc, esc_q, esc_k, sig_scale, sig_bias, num_devices=N_CORES):
    nc = bacc.Bacc("TRN2", target_bir_lowering=False, debug=False,
                   num_devices=num_devices)
    io = {
        "adj": nc.dram_tensor("adj", [rr, nn], F32, kind="ExternalInput").ap(),
        "xT": nc.dram_tensor("xT", [65, nn], BF16, kind="ExternalInput").ap(),
        "xqT": nc.dram_tensor("xqT", [65, rr], BF16, kind="ExternalInput").ap(),
        "wT": nc.dram_tensor("wT", [65, 64], BF16, kind="ExternalInput").ap(),
        "wqT": nc.dram_tensor("wqT", [64, 64], BF16, kind="ExternalInput").ap(),
        "wkT": nc.dram_tensor("wkT", [64, 64], BF16, kind="ExternalInput").ap(),
        "bq": nc.dram_tensor("bq", [1, 64], BF16, kind="ExternalInput").ap(),
        "bk": nc.dram_tensor("bk", [1, 64], BF16, kind="ExternalInput").ap(),
        "out": nc.dram_tensor("out", [rr, 64], F32, kind="ExternalOutput").ap(),
    }
    with tile.TileContext(nc) as tc:
        emit(tc, io, nn, rr, esc, esc_q, esc_k, sig_scale, sig_bias)
    nc.compile()
    return nc


def make_in_maps(inputs, nn, rr, n_cores):
    bf = ml_dtypes.bfloat16
    x = np.asarray(inputs["x"], np.float32)
    adj = np.ascontiguousarray(np.asarray(inputs["adj"], np.float32))
    W = np.asarray(inputs["W"], np.float32)
    b = np.asarray(inputs["b"], np.float32)
    Wq = np.asarray(inputs["Wq"], np.float32)
    bq = np.asarray(inputs["bq"], np.float32)
    Wk = np.asarray(inputs["Wk"], np.float32)
    bk = np.asarray(inputs["bk"], np.float32)

    xT_ext = np.concatenate([x.T, np.ones((1, nn), np.float32)], 0).astype(bf)
    wT_ext = np.concatenate([W.T, b[None, :]], 0).astype(bf)
    wqT = np.ascontiguousarray(Wq.T).astype(bf)
    wkT = np.ascontiguousarray(Wk.T).astype(bf)
    bqr = bq[None, :].astype(bf)
    bkr = bk[None, :].astype(bf)

    in_maps = []
    for c in range(n_cores):
        r0 = c * rr
        in_maps.append({
            "adj": np.ascontiguousarray(adj[r0:r0 + rr]),
            "xT": np.ascontiguousarray(xT_ext),
            "xqT": np.ascontiguousarray(xT_ext[:, r0:r0 + rr]),
            "wT": wT_ext,
            "wqT": wqT,
            "wkT": wkT,
            "bq": bqr,
            "bk": bkr,
        })
    return in_maps


def consts_from_inputs(inputs):
    scale = float(np.asarray(inputs["scale"], np.float32))
    scale_q = float(np.asarray(inputs["scale_q"], np.float32))
    scale_k = float(np.asarray(inputs["scale_k"], np.float32))
    att_bias = float(np.asarray(inputs["att_bias"], np.float32))
    att_scale = float(np.asarray(inputs["att_scale"], np.float32))
    esc = math.exp(scale)
    esc_q = math.exp(scale_q)
    esc_k = math.exp(scale_k)
    sig_scale = 2.0 / att_scale
    sig_bias = 2.0 / att_scale + att_bias
    return esc, esc_q, esc_k, sig_scale, sig_bias


def kernel(**inputs):
    nn, rr = N_FULL, R_FULL
    consts = consts_from_inputs(inputs)
    nc = build(nn, rr, *consts)
    in_maps = make_in_maps(inputs, nn, rr, N_CORES)
    res = bass_utils.run_bass_kernel_spmd(nc, in_maps,
                                          core_ids=list(range(N_CORES)))
    return np.concatenate([res.results[c]["out"] for c in range(N_CORES)],
                          axis=0)


# revision 28
# speedup vs baseline: 2.7803x; 2.7803x over previous
"""Trainium2 Bass kernel for nn_LorentzGraphConvolution.

Row-sharded across 8 NeuronCores: core c owns rows [c*1536, (c+1)*1536) of
the attention matrix / output. Every core redundantly computes the tiny
linear phase (h, k for all N; q for its local rows) from broadcast inputs,
so no collectives are needed; the only large input is each core's
[1536, 12288] slab of adj.

Key layout choices (per core):
  - att is computed TRANSPOSED (attT[j, i] tiles, j on partitions) via
    matmul(lhsT=kT block, rhs=qmT chunk) so the support matmul
    (contraction over j) consumes attT tiles directly with no transpose
    of att.
  - adj is cast f32->bf16 during the HBM DMA (SWDGE) and transposed to
    adjT in 128x128 blocks with the 2-byte xbar DMA-transpose, costing no
    engine time.
  - All matmuls run in bf16 (validated: ~8e-4 scaled output error); the
    Lorentz normalizations run in f32 on DVE/ACT from PSUM.
"""

import math
import os
import sys
from contextlib import ExitStack

for _p in ("/opt/trn_rl_repo", "/root/.axon_site/_ro/trn_rl_repo", "/root/.axon_site"):
    if os.path.isdir(_p) and _p not in sys.path:
        sys.path.insert(0, _p)

import ml_dtypes
import numpy as np

import concourse.bass as bass
import concourse.tile as tile
from concourse import bacc, bass_utils, masks, mybir

DT = mybir.dt
F32 = DT.float32
BF16 = DT.bfloat16
AF = mybir.ActivationFunctionType
ALU = mybir.AluOpType

N_FULL = 12288
D = 64
N_CORES = 8
R_FULL = N_FULL // N_CORES  # 1536 rows per core


def emit(tc, io, nn, rr, esc, esc_q, esc_k, sig_scale, sig_bias):
    """Emit the per-core Tile program.

    io: dict of bass.AP DRAM tensors:
      adj  f32  [rr, nn]      core's row slab of adj
      xT   bf16 [65, nn]      x transposed, row 64 = ones (bias row for W)
      xqT  bf16 [65, rr]      local slice of xT
      wT   bf16 [65, 64]      [W.T; b]
      wqT  bf16 [64, 64]      Wq.T
      wkT  bf16 [64, 64]      Wk.T
      bq   bf16 [1, 64]
      bk   bf16 [1, 64]
      out  f32  [rr, 64]
    """
    nc = tc.nc
    TJ = nn // 128          # global 128-row tiles
    TL = rr // 128          # local 128-row tiles
    IC = min(512, rr)       # i-chunk width (attention column block per core)
    NIC = rr // IC
    ICT = IC // 128         # 128-subtiles per i-chunk
    SW = min(2048, nn)      # adj strip width
    NSG = nn // SW
    JPG = SW // 128         # j tiles per strip group
    assert TJ % 2 == 0 and TL % 2 == 0 and rr % IC == 0 and nn % SW == 0

    ctx = ExitStack()

    const = ctx.enter_context(tc.tile_pool(name="const", bufs=1))
    persist = ctx.enter_context(tc.tile_pool(name="persist", bufs=1))
    flat = ctx.enter_context(tc.tile_pool(name="flat", bufs=2))
    psum_lin = ctx.enter_context(tc.tile_pool(name="psum_lin", bufs=2, space="PSUM"))
    psum_att = ctx.enter_context(tc.tile_pool(name="psum_att", bufs=3, space="PSUM"))
    psum_sup = ctx.enter_context(tc.tile_pool(name="psum_sup", bufs=2, space="PSUM"))
    small = ctx.enter_context(tc.tile_pool(name="small", bufs=8))
    wide = ctx.enter_context(tc.tile_pool(name="wide", bufs=2))
    oneshot = ctx.enter_context(tc.tile_pool(name="oneshot", bufs=1))
    strip_pool = ctx.enter_context(tc.tile_pool(name="strips", bufs=2 * ICT))
    adjT_pool = ctx.enter_context(tc.tile_pool(name="adjT", bufs=2))
    sig_pool = ctx.enter_context(tc.tile_pool(name="sig", bufs=4))
    attm_pool = ctx.enter_context(tc.tile_pool(name="attm", bufs=4))
    out_pool = ctx.enter_context(tc.tile_pool(name="outp", bufs=4))

    # ---- constants / small inputs -------------------------------------
    # xT shares the 2-slot "flat" pool: dead once phase A's matmuls finish,
    # freeing its slot for kpad.
    xT_s = flat.tile([65, nn], BF16, tag="flat")
    nc.sync.dma_start(xT_s[:], io["xT"][:])
    xqT_s = const.tile([65, rr], BF16)
    nc.sync.dma_start(xqT_s[:], io["xqT"][:])
    wT_s = const.tile([65, 64], BF16)
    nc.sync.dma_start(wT_s[:], io["wT"][:])
    wqT_s = const.tile([64, 64], BF16)
    nc.sync.dma_start(wqT_s[:], io["wqT"][:])
    wkT_s = const.tile([64, 64], BF16)
    nc.sync.dma_start(wkT_s[:], io["wkT"][:])
    bq_s = const.tile([1, 64], BF16)
    nc.sync.dma_start(bq_s[:], io["bq"][:])
    bk_s = const.tile([1, 64], BF16)
    nc.sync.dma_start(bk_s[:], io["bk"][:])
    ones_col = const.tile([1, 128], BF16)
    nc.vector.memset(ones_col[:], 1.0)
    ident = const.tile([64, 64], F32)
    masks.make_identity(nc, ident[:])
    sig_bias_ap = const.tile([128, 1], F32)
    nc.vector.memset(sig_bias_ap[:], sig_bias)

    # persistent per-core tensors. "pad" slabs put tile t's 64 features in
    # cols [t*128, t*128+64) so a 128x128 block DMA-transpose lands the
    # features at partitions 0:64; pad regions are never read.
    hpad = persist.tile([128, TJ * 128], BF16)      # h, natural row tiles
    kT_flat = persist.tile([128, TJ * 128], BF16)   # k^T, rows 0:64 valid
    qmT_flat = persist.tile([128, TL * 128], BF16)  # qm^T, rows 0:64 valid

    hpad3 = hpad.rearrange("p (t c) -> p t c", c=128)
    nc.vector.memset(hpad[:], 0.0)

    # ---- batched LorentzLinear ---------------------------------------
    def lorentz_linear(tiles, lhsT_fn, rhs_w, bias_row, esc_, neg, wr_slab3, pad):
        """Matmul + Lorentz renormalization for a batch of row tiles.

        wr_slab3: [128, T, c] destination view (c = 64 dense or 128 padded);
        writes time into [:, t, 0] and scaled spatial into [:, t, 1:64].
        """
        nb = len(tiles)
        ps = psum_lin.tile([128, nb * 64], F32, tag="linpsum")
        ps3 = ps.rearrange("p (t d) -> p t d", d=64)
        for u, t in enumerate(tiles):
            o = ps[:, u * 64:(u + 1) * 64]
            if bias_row is None:
                nc.tensor.matmul(o, lhsT_fn(t), rhs_w, start=True, stop=True)
            else:
                nc.tensor.matmul(o, lhsT_fn(t), rhs_w, start=True, stop=False)
                nc.tensor.matmul(o, ones_col[:], bias_row, start=False, stop=True)
        sg = small.tile([128, nb], F32, tag="nsg")
        nc.scalar.activation(sg[:], ps3[:, :, 0], AF.Sigmoid)
        time = small.tile([128, nb], F32, tag="ntime")
        a, c0 = (-esc_, -1.1) if neg else (esc_, 1.1)
        nc.vector.tensor_scalar(time[:], sg[:], a, c0, ALU.mult, ALU.add)
        sqf = wide.tile([128, nb * 64], F32, tag="nsqf")
        nc.scalar.activation(sqf[:], ps[:], AF.Square)
        sqf3 = sqf.rearrange("p (t d) -> p t d", d=64)
        tot = small.tile([128, nb], F32, tag="ntot")
        nc.vector.tensor_reduce(tot[:], sqf3[:], axis=mybir.AxisListType.X,
                                op=ALU.add)
        p0sq = small.tile([128, nb], F32, tag="np0")
        nc.vector.tensor_copy(p0sq[:], sqf3[:, :, 0])
        sq = small.tile([128, nb], F32, tag="nsq")
        # sq = tot - p0sq  (spatial sum of squares)
        nc.vector.scalar_tensor_tensor(sq[:], p0sq[:], -1.0, tot[:],
                                       ALU.mult, ALU.add)
        sqc = small.tile([128, nb], F32, tag="nsqc")
        nc.vector.tensor_scalar_max(sqc[:], sq[:], 1e-8)
        rcp = small.tile([128, nb], F32, tag="nrcp")
        nc.vector.reciprocal(rcp[:], sqc[:])
        t2 = small.tile([128, nb], F32, tag="nt2")
        nc.vector.tensor_tensor(t2[:], time[:], time[:], ALU.mult)
        sval = small.tile([128, nb], F32, tag="nsv")
        # s = (t^2 - 1) * rcp
        nc.vector.scalar_tensor_tensor(sval[:], t2[:], -1.0, rcp[:],
                                       ALU.add, ALU.mult)
        sqs = small.tile([128, nb], F32, tag="nsqs")
        nc.scalar.activation(sqs[:], sval[:], AF.Sqrt)
        t0 = tiles[0]
        nc.vector.tensor_copy(wr_slab3[:, t0:t0 + nb, 0], time[:])
        for u, t in enumerate(tiles):
            nc.vector.tensor_scalar_mul(wr_slab3[:, t, 1:64],
                                        ps3[:, u, 1:64], sqs[:, u:u + 1])

    def batches(total):
        return [list(range(s, min(s + 8, total))) for s in range(0, total, 8)]

    # ---- phase A: h (all rows) ---------------------------------------
    for bt in batches(TJ):
        lorentz_linear(bt, lambda t: xT_s[:, t * 128:(t + 1) * 128],
                       wT_s[:], None, esc, False, hpad3, False)
    # One xbar instruction transposes every 128x128 block: with a 3D out AP
    # [128, T, 128], out[a, t, n] = in[n, t*128 + a] -- per-block transpose.
    hT_flat = flat.tile([128, TJ * 128], BF16, tag="flat")
    nc.sync.dma_start(hT_flat.rearrange("p (t n) -> p t n", n=128),
                      hpad[:], transpose=True)

    # ---- phase B: k (all rows) ---------------------------------------
    kpad = flat.tile([128, TJ * 128], BF16, tag="flat")
    kpad3 = kpad.rearrange("p (t c) -> p t c", c=128)
    nc.vector.memset(kpad[:], 0.0)

    def h_lhsT(t):
        return hT_flat[0:64, t * 128:(t + 1) * 128]

    for bt in batches(TJ):
        lorentz_linear(bt, h_lhsT, wkT_s[:], bk_s[:], esc_k, False,
                       kpad3, False)
    nc.sync.dma_start(kT_flat.rearrange("p (t n) -> p t n", n=128),
                      kpad[:], transpose=True)

    # ---- phase Bq: hq + qm (local rows) ------------------------------
    hqpad = oneshot.tile([128, TL * 128], BF16, tag="hq")
    hqpad3 = hqpad.rearrange("p (t c) -> p t c", c=128)
    nc.vector.memset(hqpad[:], 0.0)
    for bt in batches(TL):
        lorentz_linear(bt, lambda t: xqT_s[:, t * 128:(t + 1) * 128],
                       wT_s[:], None, esc, False, hqpad3, False)
    hqT_flat = oneshot.tile([128, TL * 128], BF16, tag="hqT")
    nc.sync.dma_start(hqT_flat.rearrange("p (t n) -> p t n", n=128),
                      hqpad[:], transpose=True)

    qm_pad = oneshot.tile([128, TL * 128], BF16, tag="qmpad")
    qm_pad3 = qm_pad.rearrange("p (t c) -> p t c", c=128)
    nc.vector.memset(qm_pad[:], 0.0)

    def hq_lhsT(t):
        return hqT_flat[0:64, t * 128:(t + 1) * 128]

    for bt in batches(TL):
        lorentz_linear(bt, hq_lhsT, wqT_s[:], bq_s[:], esc_q, True,
                       qm_pad3, True)
    nc.sync.dma_start(qmT_flat.rearrange("p (t n) -> p t n", n=128),
                      qm_pad[:], transpose=True)

    # ---- phase C: attention + support --------------------------------
    for c in range(NIC):
        supT = psum_sup.tile([64, IC], F32, tag="supT")
        for g in range(NSG):
            strips = []
            for s in range(ICT):
                st = strip_pool.tile([128, SW], BF16, tag="strip")
                r0 = c * IC + s * 128
                nc.gpsimd.dma_start(st[:], io["adj"][r0:r0 + 128,
                                                     g * SW:(g + 1) * SW])
                strips.append(st)
            # adjT for the whole strip group in ICT transpose instructions:
            # strip s (adj rows r0+s*128..+128, cols g*SW..) block-transposes
            # into [j-within-tile, jl, i] slices of the group tile.
            adjTg = adjT_pool.tile([128, JPG * IC], BF16, tag="adjT")
            adjTg3 = adjTg.rearrange("p (t i) -> p t i", i=IC)
            for s in range(ICT):
                nc.sync.dma_start(adjTg3[:, :, s * 128:(s + 1) * 128],
                                  strips[s][:], transpose=True)
            for jl in range(JPG):
                j = g * JPG + jl
                adjT = adjTg[:, jl * IC:(jl + 1) * IC]
                attT = psum_att.tile([128, IC], F32, tag="attT")
                lhsT_k = kT_flat[0:64, j * 128:(j + 1) * 128]
                nc.tensor.matmul(attT[:], lhsT_k,
                                 qmT_flat[0:64, c * IC:(c + 1) * IC],
                                 start=True, stop=True)
                sig = sig_pool.tile([128, IC], BF16, tag="sig")
                nc.scalar.activation(sig[:], attT[:], AF.Sigmoid,
                                     bias=sig_bias_ap[:], scale=sig_scale)
                attm = attm_pool.tile([128, IC], BF16, tag="attm")
                nc.vector.tensor_mul(attm[:], sig[:], adjT)
                nc.tensor.matmul(supT[:], hpad[:, j * 128:j * 128 + 64],
                                 attm[:], start=(j == 0), stop=(j == TJ - 1))
        # normalize + write out this i-chunk
        supTs = wide.tile([64, IC], F32, tag="supTs")
        nc.vector.tensor_copy(supTs[:], supT[:])
        for s in range(ICT):
            supn = psum_lin.tile([128, 64], F32, tag="linpsum")
            nc.tensor.transpose(supn[:], supTs[:, s * 128:(s + 1) * 128],
                                ident[:])
            sq64 = out_pool.tile([128, 64], F32, tag="sq64")
            nc.scalar.activation(sq64[:], supn[:], AF.Square)
            tot = small.tile([128, 1], F32, tag="ftot")
            nc.vector.tensor_reduce(tot[:], sq64[:], axis=mybir.AxisListType.X,
                                    op=ALU.add)
            inner = small.tile([128, 1], F32, tag="finner")
            # inner = tot - 2*s0^2  (= -s0^2 + sum_{d>=1} s_d^2)
            nc.vector.scalar_tensor_tensor(inner[:], sq64[:, 0:1], -2.0,
                                           tot[:], ALU.mult, ALU.add)
            negv = small.tile([128, 1], F32, tag="fneg")
            nc.vector.tensor_scalar_mul(negv[:], inner[:], -1.0)
            absv = small.tile([128, 1], F32, tag="fabs")
            nc.vector.tensor_tensor(absv[:], inner[:], negv[:], ALU.max)
            clipv = small.tile([128, 1], F32, tag="fclip")
            nc.vector.tensor_scalar_max(clipv[:], absv[:], 1e-8)
            rcp = small.tile([128, 1], F32, tag="frcp")
            nc.vector.reciprocal(rcp[:], clipv[:])
            rs = small.tile([128, 1], F32, tag="frs")
            nc.scalar.activation(rs[:], rcp[:], AF.Sqrt)
            o = out_pool.tile([128, 64], F32, tag="otile")
            nc.vector.tensor_scalar_mul(o[:], supn[:], rs[:])
            r0 = c * IC + s * 128
            nc.sync.dma_start(io["out"][r0:r0 + 128, :], o[:])

    ctx.close()


def build(nn, rr, esc, esc_q, esc_k, sig_scale, sig_bias, num_devices=N_CORES):
    nc = bacc.Bacc("TRN2", target_bir_lowering=False, debug=False,
                   num_devices=num_devices)
    io = {
        "adj": nc.dram_tensor("adj", [rr, nn], F32, kind="ExternalInput").ap(),
        "xT": nc.dram_tensor("xT", [65, nn], BF16, kind="ExternalInput").ap(),
        "xqT": nc.dram_tensor("xqT", [65, rr], BF16, kind="ExternalInput").ap(),
        "wT": nc.dram_tensor("wT", [65, 64], BF16, kind="ExternalInput").ap(),
        "wqT": nc.dram_tensor("wqT", [64, 64], BF16, kind="ExternalInput").ap(),
        "wkT": nc.dram_tensor("wkT", [64, 64], BF16, kind="ExternalInput").ap(),
        "bq": nc.dram_tensor("bq", [1, 64], BF16, kind="ExternalInput").ap(),
        "bk": nc.dram_tensor("bk", [1, 64], BF16, kind="ExternalInput").ap(),
        "out": nc.dram_tensor("out", [rr, 64], F32, kind="ExternalOutput").ap(),
    }
    with tile.TileContext(nc) as tc:
        emit(tc, io, nn, rr, esc, esc_q, esc_k, sig_scale, sig_bias)
    nc.compile()
    return nc


def make_in_maps(inputs, nn, rr, n_cores):
    bf = ml_dtypes.bfloat16
    x = np.asarray(inputs["x"], np.float32)
    adj = np.ascontiguousarray(np.asarray(inputs["adj"], np.float32))
    W = np.asarray(inputs["W"], np.float32)
    b = np.asarray(inputs["b"], np.float32)
    Wq = np.asarray(inputs["Wq"], np.float32)
    bq = np.asarray(inputs["bq"], np.float32)
    Wk = np.asarray(inputs["Wk"], np.float32)
    bk = np.asarray(inputs["bk"], np.float32)

    xT_ext = np.concatenate([x.T, np.ones((1, nn), np.float32)], 0).astype(bf)
    wT_ext = np.concatenate([W.T, b[None, :]], 0).astype(bf)
    wqT = np.ascontiguousarray(Wq.T).astype(bf)
    wkT = np.ascontiguousarray(Wk.T).astype(bf)
    bqr = bq[None, :].astype(bf)
    bkr = bk[None, :].astype(bf)

    in_maps = []
    for c in range(n_cores):
        r0 = c * rr
        in_maps.append({
            "adj": np.ascontiguousarray(adj[r0:r0 + rr]),
            "xT": np.ascontiguousarray(xT_ext),
            "xqT": np.ascontiguousarray(xT_ext[:, r0:r0 + rr]),
            "wT": wT_ext,
            "wqT": wqT,
            "wkT": wkT,
            "bq": bqr,
            "bk": bkr,
        })
    return in_maps


def consts_from_inputs(inputs):
    scale = float(np.asarray(inputs["scale"], np.float32))
    scale_q = float(np.asarray(inputs["scale_q"], np.float32))
    scale_k = float(np.asarray(inputs["scale_k"], np.float32))
    att_bias = float(np.asarray(inputs["att_bias"], np.float32))
    att_scale = float(np.asarray(inputs["att_scale"], np.float32))
    esc = math.exp(scale)
    esc_q = math.exp(scale_q)
    esc_k = math.exp(scale_k)
    sig_scale = 2.0 / att_scale
    sig_bias = 2.0 / att_scale + att_bias
    return esc, esc_q, esc_k, sig_scale, sig_bias


def kernel(**inputs):
    nn, rr = N_FULL, R_FULL
    consts = consts_from_inputs(inputs)
    nc = build(nn, rr, *consts)
    in_maps = make_in_maps(inputs, nn, rr, N_CORES)
    res = bass_utils.run_bass_kernel_spmd(nc, in_maps,
                                          core_ids=list(range(N_CORES)))
    return np.concatenate([res.results[c]["out"] for c in range(N_CORES)],
                          axis=0)


# revision 33
# speedup vs baseline: 3.3322x; 1.1985x over previous
"""Trainium2 Bass kernel for nn_LorentzGraphConvolution.

Row-sharded across 8 NeuronCores: core c owns rows [c*1536, (c+1)*1536) of
the attention matrix / output. Every core redundantly computes the tiny
linear phase (h, k for all N; q for its local rows) from broadcast inputs,
so no collectives are needed; the only large input is each core's
[1536, 12288] slab of adj.

Key layout choices (per core):
  - att is computed TRANSPOSED (attT[j, i] tiles, j on partitions) via
    matmul(lhsT=kT block, rhs=qmT chunk) so the support matmul
    (contraction over j) consumes attT tiles directly with no transpose
    of att.
  - adj is cast f32->bf16 during the HBM DMA (SWDGE) and transposed to
    adjT in 128x128 blocks with the 2-byte xbar DMA-transpose, costing no
    engine time.
  - All matmuls run in bf16 (validated: ~8e-4 scaled output error); the
    Lorentz normalizations run in f32 on DVE/ACT from PSUM.
"""

import math
import os
import sys
from contextlib import ExitStack

for _p in ("/opt/trn_rl_repo", "/root/.axon_site/_ro/trn_rl_repo", "/root/.axon_site"):
    if os.path.isdir(_p) and _p not in sys.path:
        sys.path.insert(0, _p)

import ml_dtypes
import numpy as np

import concourse.bass as bass
import concourse.tile as tile
from concourse import bacc, bass_utils, masks, mybir

DT = mybir.dt
F32 = DT.float32
BF16 = DT.bfloat16
AF = mybir.ActivationFunctionType
ALU = mybir.AluOpType

N_FULL = 12288
D = 64
N_CORES = 8
R_FULL = N_FULL // N_CORES  # 1536 rows per core


def emit(tc, io, nn, rr, esc, esc_q, esc_k, sig_scale, sig_bias):
    # Additive masking: attT psum accumulates BIG*adjT via PE
    # transpose-matmuls (lhsT=adj block, rhs=BIG*I); the sigmoid bias then
    # subtracts BIG*sig_scale so adj=1 entries are exact and adj=0 entries
    # give sigmoid(<= -25) ~ 1e-11 (negligible vs the true values).
    import ml_dtypes as _mld
    BIG = float(np.float32(_mld.bfloat16(45.0 / sig_scale)))
    """Emit the per-core Tile program.

    io: dict of bass.AP DRAM tensors:
      adj  f32  [rr, nn]      core's row slab of adj
      xT   bf16 [65, nn]      x transposed, row 64 = ones (bias row for W)
      xqT  bf16 [65, rr]      local slice of xT
      wT   bf16 [65, 64]      [W.T; b]
      wqT  bf16 [64, 64]      Wq.T
      wkT  bf16 [64, 64]      Wk.T
      bq   bf16 [1, 64]
      bk   bf16 [1, 64]
      out  f32  [rr, 64]
    """
    nc = tc.nc
    TJ = nn // 128          # global 128-row tiles
    TL = rr // 128          # local 128-row tiles
    IC = min(512, rr)       # i-chunk width (attention column block per core)
    NIC = rr // IC
    ICT = IC // 128         # 128-subtiles per i-chunk
    SW = min(2048, nn)      # adj strip width
    NSG = nn // SW
    JPG = SW // 128         # j tiles per strip group
    assert TJ % 2 == 0 and TL % 2 == 0 and rr % IC == 0 and nn % SW == 0

    ctx = ExitStack()

    const = ctx.enter_context(tc.tile_pool(name="const", bufs=1))
    persist = ctx.enter_context(tc.tile_pool(name="persist", bufs=1))
    flat = ctx.enter_context(tc.tile_pool(name="flat", bufs=2))
    psum_lin = ctx.enter_context(tc.tile_pool(name="psum_lin", bufs=2, space="PSUM"))
    psum_att = ctx.enter_context(tc.tile_pool(name="psum_att", bufs=3, space="PSUM"))
    psum_sup = ctx.enter_context(tc.tile_pool(name="psum_sup", bufs=2, space="PSUM"))
    small = ctx.enter_context(tc.tile_pool(name="small", bufs=8))
    wide = ctx.enter_context(tc.tile_pool(name="wide", bufs=2))
    oneshot = ctx.enter_context(tc.tile_pool(name="oneshot", bufs=1))
    strip_pool = ctx.enter_context(tc.tile_pool(name="strips", bufs=2 * ICT))
    sig_pool = ctx.enter_context(tc.tile_pool(name="sig", bufs=6))
    out_pool = ctx.enter_context(tc.tile_pool(name="outp", bufs=4))

    # ---- constants / small inputs -------------------------------------
    # xT shares the 2-slot "flat" pool: dead once phase A's matmuls finish,
    # freeing its slot for kpad.
    xT_s = flat.tile([65, nn], BF16, tag="flat")
    nc.sync.dma_start(xT_s[:], io["xT"][:])
    xqT_s = const.tile([65, rr], BF16)
    nc.sync.dma_start(xqT_s[:], io["xqT"][:])
    wT_s = const.tile([65, 64], BF16)
    nc.sync.dma_start(wT_s[:], io["wT"][:])
    wqT_s = const.tile([64, 64], BF16)
    nc.sync.dma_start(wqT_s[:], io["wqT"][:])
    wkT_s = const.tile([64, 64], BF16)
    nc.sync.dma_start(wkT_s[:], io["wkT"][:])
    bq_s = const.tile([1, 64], BF16)
    nc.sync.dma_start(bq_s[:], io["bq"][:])
    bk_s = const.tile([1, 64], BF16)
    nc.sync.dma_start(bk_s[:], io["bk"][:])
    ones_col = const.tile([1, 128], BF16)
    nc.vector.memset(ones_col[:], 1.0)
    ident = const.tile([64, 64], F32)
    masks.make_identity(nc, ident[:])
    sig_bias_ap = const.tile([128, 1], F32)
    nc.vector.memset(sig_bias_ap[:], sig_bias - BIG * sig_scale)
    bigI = const.tile([128, 128], BF16)
    nc.gpsimd.memset(bigI[:], 0.0)
    nc.gpsimd.affine_select(
        out=bigI[:], in_=bigI[:], compare_op=ALU.not_equal, fill=BIG,
        base=0, pattern=[[-1, 128]], channel_multiplier=1)

    # persistent per-core tensors. "pad" slabs put tile t's 64 features in
    # cols [t*128, t*128+64) so a 128x128 block DMA-transpose lands the
    # features at partitions 0:64; pad regions are never read.
    hpad = persist.tile([128, TJ * 128], BF16)      # h, natural row tiles
    kT_flat = persist.tile([128, TJ * 128], BF16)   # k^T, rows 0:64 valid
    qmT_flat = persist.tile([128, TL * 128], BF16)  # qm^T, rows 0:64 valid

    hpad3 = hpad.rearrange("p (t c) -> p t c", c=128)
    nc.vector.memset(hpad[:], 0.0)

    # ---- batched LorentzLinear ---------------------------------------
    def lorentz_linear(tiles, lhsT_fn, rhs_w, bias_row, esc_, neg, wr_slab3, pad):
        """Matmul + Lorentz renormalization for a batch of row tiles.

        wr_slab3: [128, T, c] destination view (c = 64 dense or 128 padded);
        writes time into [:, t, 0] and scaled spatial into [:, t, 1:64].
        """
        nb = len(tiles)
        ps = psum_lin.tile([128, nb * 64], F32, tag="linpsum")
        ps3 = ps.rearrange("p (t d) -> p t d", d=64)
        for u, t in enumerate(tiles):
            o = ps[:, u * 64:(u + 1) * 64]
            if bias_row is None:
                nc.tensor.matmul(o, lhsT_fn(t), rhs_w, start=True, stop=True)
            else:
                nc.tensor.matmul(o, lhsT_fn(t), rhs_w, start=True, stop=False)
                nc.tensor.matmul(o, ones_col[:], bias_row, start=False, stop=True)
        sg = small.tile([128, nb], F32, tag="nsg")
        nc.scalar.activation(sg[:], ps3[:, :, 0], AF.Sigmoid)
        time = small.tile([128, nb], F32, tag="ntime")
        a, c0 = (-esc_, -1.1) if neg else (esc_, 1.1)
        nc.vector.tensor_scalar(time[:], sg[:], a, c0, ALU.mult, ALU.add)
        sqf = wide.tile([128, nb * 64], F32, tag="nsqf")
        nc.scalar.activation(sqf[:], ps[:], AF.Square)
        sqf3 = sqf.rearrange("p (t d) -> p t d", d=64)
        tot = small.tile([128, nb], F32, tag="ntot")
        nc.vector.tensor_reduce(tot[:], sqf3[:], axis=mybir.AxisListType.X,
                                op=ALU.add)
        p0sq = small.tile([128, nb], F32, tag="np0")
        nc.vector.tensor_copy(p0sq[:], sqf3[:, :, 0])
        sq = small.tile([128, nb], F32, tag="nsq")
        # sq = tot - p0sq  (spatial sum of squares)
        nc.vector.scalar_tensor_tensor(sq[:], p0sq[:], -1.0, tot[:],
                                       ALU.mult, ALU.add)
        sqc = small.tile([128, nb], F32, tag="nsqc")
        nc.vector.tensor_scalar_max(sqc[:], sq[:], 1e-8)
        rcp = small.tile([128, nb], F32, tag="nrcp")
        nc.vector.reciprocal(rcp[:], sqc[:])
        t2 = small.tile([128, nb], F32, tag="nt2")
        nc.vector.tensor_tensor(t2[:], time[:], time[:], ALU.mult)
        sval = small.tile([128, nb], F32, tag="nsv")
        # s = (t^2 - 1) * rcp
        nc.vector.scalar_tensor_tensor(sval[:], t2[:], -1.0, rcp[:],
                                       ALU.add, ALU.mult)
        sqs = small.tile([128, nb], F32, tag="nsqs")
        nc.scalar.activation(sqs[:], sval[:], AF.Sqrt)
        t0 = tiles[0]
        nc.vector.tensor_copy(wr_slab3[:, t0:t0 + nb, 0], time[:])
        for u, t in enumerate(tiles):
            nc.vector.tensor_scalar_mul(wr_slab3[:, t, 1:64],
                                        ps3[:, u, 1:64], sqs[:, u:u + 1])

    def batches(total):
        return [list(range(s, min(s + 8, total))) for s in range(0, total, 8)]

    # ---- phase A: h (all rows) ---------------------------------------
    for bt in batches(TJ):
        lorentz_linear(bt, lambda t: xT_s[:, t * 128:(t + 1) * 128],
                       wT_s[:], None, esc, False, hpad3, False)
    # One xbar instruction transposes every 128x128 block: with a 3D out AP
    # [128, T, 128], out[a, t, n] = in[n, t*128 + a] -- per-block transpose.
    hT_flat = flat.tile([128, TJ * 128], BF16, tag="flat")
    nc.sync.dma_start(hT_flat.rearrange("p (t n) -> p t n", n=128),
                      hpad[:], transpose=True)

    # ---- phase B: k (all rows) ---------------------------------------
    kpad = flat.tile([128, TJ * 128], BF16, tag="flat")
    kpad3 = kpad.rearrange("p (t c) -> p t c", c=128)
    nc.vector.memset(kpad[:], 0.0)

    def h_lhsT(t):
        return hT_flat[0:64, t * 128:(t + 1) * 128]

    for bt in batches(TJ):
        lorentz_linear(bt, h_lhsT, wkT_s[:], bk_s[:], esc_k, False,
                       kpad3, False)
    nc.sync.dma_start(kT_flat.rearrange("p (t n) -> p t n", n=128),
                      kpad[:], transpose=True)

    # ---- phase Bq: hq + qm (local rows) ------------------------------
    hqpad = oneshot.tile([128, TL * 128], BF16, tag="hq")
    hqpad3 = hqpad.rearrange("p (t c) -> p t c", c=128)
    nc.vector.memset(hqpad[:], 0.0)
    for bt in batches(TL):
        lorentz_linear(bt, lambda t: xqT_s[:, t * 128:(t + 1) * 128],
                       wT_s[:], None, esc, False, hqpad3, False)
    hqT_flat = oneshot.tile([128, TL * 128], BF16, tag="hqT")
    nc.sync.dma_start(hqT_flat.rearrange("p (t n) -> p t n", n=128),
                      hqpad[:], transpose=True)

    qm_pad = oneshot.tile([128, TL * 128], BF16, tag="qmpad")
    qm_pad3 = qm_pad.rearrange("p (t c) -> p t c", c=128)
    nc.vector.memset(qm_pad[:], 0.0)

    def hq_lhsT(t):
        return hqT_flat[0:64, t * 128:(t + 1) * 128]

    for bt in batches(TL):
        lorentz_linear(bt, hq_lhsT, wqT_s[:], bq_s[:], esc_q, True,
                       qm_pad3, True)
    nc.sync.dma_start(qmT_flat.rearrange("p (t n) -> p t n", n=128),
                      qm_pad[:], transpose=True)

    # ---- phase C: attention + support --------------------------------
    for c in range(NIC):
        supT = psum_sup.tile([64, IC], F32, tag="supT")
        for g in range(NSG):
            strips = []
            for s in range(ICT):
                st = strip_pool.tile([128, SW], BF16, tag="strip")
                r0 = c * IC + s * 128
                nc.gpsimd.dma_start(st[:], io["adj"][r0:r0 + 128,
                                                     g * SW:(g + 1) * SW])
                strips.append(st)
            for jl in range(JPG):
                j = g * JPG + jl
                attT = psum_att.tile([128, IC], F32, tag="attT")
                lhsT_k = kT_flat[0:64, j * 128:(j + 1) * 128]
                nc.tensor.matmul(attT[:], lhsT_k,
                                 qmT_flat[0:64, c * IC:(c + 1) * IC],
                                 start=True, stop=False)
                # accumulate BIG*adjT into the same bank: PE-transposed
                # adj blocks (out[jf, i] += BIG * adj[i, j*128+jf])
                for s in range(ICT):
                    nc.tensor.matmul(attT[:, s * 128:(s + 1) * 128],
                                     strips[s][:, jl * 128:(jl + 1) * 128],
                                     bigI[:], start=False, stop=True)
                sig = sig_pool.tile([128, IC], BF16, tag="sig")
                nc.scalar.activation(sig[:], attT[:], AF.Sigmoid,
                                     bias=sig_bias_ap[:], scale=sig_scale)
                nc.tensor.matmul(supT[:], hpad[:, j * 128:j * 128 + 64],
                                 sig[:], start=(j == 0), stop=(j == TJ - 1))
        # normalize + write out this i-chunk
        supTs = wide.tile([64, IC], F32, tag="supTs")
        nc.vector.tensor_copy(supTs[:], supT[:])
        for s in range(ICT):
            supn = psum_lin.tile([128, 64], F32, tag="linpsum")
            nc.tensor.transpose(supn[:], supTs[:, s * 128:(s + 1) * 128],
                                ident[:])
            sq64 = out_pool.tile([128, 64], F32, tag="sq64")
            nc.scalar.activation(sq64[:], supn[:], AF.Square)
            tot = small.tile([128, 1], F32, tag="ftot")
            nc.vector.tensor_reduce(tot[:], sq64[:], axis=mybir.AxisListType.X,
                                    op=ALU.add)
            inner = small.tile([128, 1], F32, tag="finner")
            # inner = tot - 2*s0^2  (= -s0^2 + sum_{d>=1} s_d^2)
            nc.vector.scalar_tensor_tensor(inner[:], sq64[:, 0:1], -2.0,
                                           tot[:], ALU.mult, ALU.add)
            negv = small.tile([128, 1], F32, tag="fneg")
            nc.vector.tensor_scalar_mul(negv[:], inner[:], -1.0)
            absv = small.tile([128, 1], F32, tag="fabs")
            nc.vector.tensor_tensor(absv[:], inner[:], negv[:], ALU.max)
            clipv = small.tile([128, 1], F32, tag="fclip")
            nc.vector.tensor_scalar_max(clipv[:], absv[:], 1e-8)
            rcp = small.tile([128, 1], F32, tag="frcp")
            nc.vector.reciprocal(rcp[:], clipv[:])
            rs = small.tile([128, 1], F32, tag="frs")
            nc.scalar.activation(rs[:], rcp[:], AF.Sqrt)
            o = out_pool.tile([128, 64], F32, tag="otile")
            nc.vector.tensor_scalar_mul(o[:], supn[:], rs[:])
            r0 = c * IC + s * 128
            nc.sync.dma_start(io["out"][r0:r0 + 128, :], o[:])

    ctx.close()


def build(nn, rr, esc, esc_q, esc_k, sig_scale, sig_bias, num_devices=N_CORES):
    nc = bacc.Bacc("TRN2", target_bir_lowering=False, debug=False,
                   num_devices=num_devices)
    io = {
        "adj": nc.dram_tensor("adj", [rr, nn], F32, kind="ExternalInput").ap(),
        "xT": nc.dram_tensor("xT", [65, nn], BF16, kind="ExternalInput").ap(),
        "xqT": nc.dram_tensor("xqT", [65, rr], BF16, kind="ExternalInput").ap(),
        "wT": nc.dram_tensor("wT", [65, 64], BF16, kind="ExternalInput").ap(),
        "wqT": nc.dram_tensor("wqT", [64, 64], BF16, kind="ExternalInput").ap(),
        "wkT": nc.dram_tensor("wkT", [64, 64], BF16, kind="ExternalInput").ap(),
        "bq": nc.dram_tensor("bq", [1, 64], BF16, kind="ExternalInput").ap(),
        "bk": nc.dram_tensor("bk", [1, 64], BF16, kind="ExternalInput").ap(),
        "out": nc.dram_tensor("out", [rr, 64], F32, kind="ExternalOutput").ap(),
    }
    with tile.TileContext(nc) as tc:
        emit(tc, io, nn, rr, esc, esc_q, esc_k, sig_scale, sig_bias)
    nc.compile()
    return nc


def make_in_maps(inputs, nn, rr, n_cores):
    bf = ml_dtypes.bfloat16
    x = np.asarray(inputs["x"], np.float32)
    adj = np.ascontiguousarray(np.asarray(inputs["adj"], np.float32))
    W = np.asarray(inputs["W"], np.float32)
    b = np.asarray(inputs["b"], np.float32)
    Wq = np.asarray(inputs["Wq"], np.float32)
    bq = np.asarray(inputs["bq"], np.float32)
    Wk = np.asarray(inputs["Wk"], np.float32)
    bk = np.asarray(inputs["bk"], np.float32)

    xT_ext = np.concatenate([x.T, np.ones((1, nn), np.float32)], 0).astype(bf)
    wT_ext = np.concatenate([W.T, b[None, :]], 0).astype(bf)
    wqT = np.ascontiguousarray(Wq.T).astype(bf)
    wkT = np.ascontiguousarray(Wk.T).astype(bf)
    bqr = bq[None, :].astype(bf)
    bkr = bk[None, :].astype(bf)

    in_maps = []
    for c in range(n_cores):
        r0 = c * rr
        in_maps.append({
            "adj": np.ascontiguousarray(adj[r0:r0 + rr]),
            "xT": np.ascontiguousarray(xT_ext),
            "xqT": np.ascontiguousarray(xT_ext[:, r0:r0 + rr]),
            "wT": wT_ext,
            "wqT": wqT,
            "wkT": wkT,
            "bq": bqr,
            "bk": bkr,
        })
    return in_maps


def consts_from_inputs(inputs):
    scale = float(np.asarray(inputs["scale"], np.float32))
    scale_q = float(np.asarray(inputs["scale_q"], np.float32))
    scale_k = float(np.asarray(inputs["scale_k"], np.float32))
    att_bias = float(np.asarray(inputs["att_bias"], np.float32))
    att_scale = float(np.asarray(inputs["att_scale"], np.float32))
    esc = math.exp(scale)
    esc_q = math.exp(scale_q)
    esc_k = math.exp(scale_k)
    sig_scale = 2.0 / att_scale
    sig_bias = 2.0 / att_scale + att_bias
    return esc, esc_q, esc_k, sig_scale, sig_bias


def kernel(**inputs):
    nn, rr = N_FULL, R_FULL
    consts = consts_from_inputs(inputs)
    nc = build(nn, rr, *consts)
    in_maps = make_in_maps(inputs, nn, rr, N_CORES)
    res = bass_utils.run_bass_kernel_spmd(nc, in_maps,
                                          core_ids=list(range(N_CORES)))
    return np.concatenate([res.results[c]["out"] for c in range(N_CORES)],
                          axis=0)
